# revision 41
# baseline (speedup 1.0000x reference)
"""Trainium2 Bass kernel for nn_AttnBlock: LayerNorm -> 16-head attention -> out-proj.

Full inputs in, full output out. Sharding: 8 cores = 2 batches x 4 head-groups
(4 heads per core). Each core computes LN + QKV (its 256 feature slice) +
attention for its 4 heads + a partial output projection; the host sums the 4
partials per batch and adds the output bias.

v3 changes vs baseline (335 us):
  - LN stats off the tensor engine: x also arrives token-major (xtok) and the
    stats run on the vector engine while the PE runs Q/K chunks 0..7 (which
    don't depend on the stats). LN chain in token-major [128, 16] form
    (short ops) instead of [1, 2048] rows.
  - Q/K head-pair 0 accumulates chunk-outer across 8 PSUM banks so the PE
    starts as soon as the first x^T chunk lands; the augmented chunk 8
    attaches once the stats bounce completes.
  - attention in 512-token q slabs with BOTH heads of a pair packed into one
    [128, 1024] score tile (ho0 -> cols 0:512, ho1 -> 512:1024; the two K=64
    score matmuls still run concurrently in disjoint PE row groups), so one
    1024-wide exp covers the pair -> half the ScalarE instruction overhead.
  - AV software-pipelined one k-tile behind the scores so the PE never waits
    on the exp.
  - out-proj of slab q runs one slab later (inside slab q+1's PE stream),
    hiding the normalize latency and shrinking the end tail.
  - weights go first on the sync queue, xtok first on the scalar queue, so
    Q/K weights precede x^T and the stats inputs are not starved.
"""

import os
from contextlib import ExitStack

import numpy as np

import concourse.bass as bass
import concourse.tile as tile
from concourse import bacc, mybir
from concourse.bass_utils import run_bass_kernel_spmd

F32 = mybir.dt.float32
BF16 = mybir.dt.bfloat16

B, L, D = 2, 2048, 1024
NH_TOT, HS = 16, 64
NCORES = 8
HPC = 4                  # heads per core
FPC = HPC * HS           # 256 features per core
P = 128
DCH = D // P             # 8 x^T chunks
KCH = DCH + 1            # +1 augmented chunk
QS = 512                 # q slab
NQS = L // QS            # 4
KT = L // P              # 16 k tiles
TT = L // P              # 16 token tiles
EPS = 1e-5
SCALE = float(HS) ** -0.5

LAST_RESULTS = None


def _build_nc():
    nc = bacc.Bacc("TRN2", target_bir_lowering=False, debug=False)

    xT = nc.dram_tensor("xT", [D, L], BF16, kind="ExternalInput").ap()
    xtok = nc.dram_tensor("xtok", [L, D], BF16, kind="ExternalInput").ap()
    wq = nc.dram_tensor("wq", [KCH * P, FPC], BF16, kind="ExternalInput").ap()
    wk = nc.dram_tensor("wk", [KCH * P, FPC], BF16, kind="ExternalInput").ap()
    wv = nc.dram_tensor("wv", [KCH * P, FPC], BF16, kind="ExternalInput").ap()
    wo = nc.dram_tensor("wo", [FPC, D], BF16, kind="ExternalInput").ap()
    ident = nc.dram_tensor("ident", [P, P], F32, kind="ExternalInput").ap()
    out = nc.dram_tensor("out", [L, D], BF16, kind="ExternalOutput").ap()

    with tile.TileContext(nc) as tc, ExitStack() as ctx:
        persist = ctx.enter_context(tc.tile_pool(name="persist", bufs=1))

        # ---------------- persistent tiles ----------------
        eps_t = persist.tile([P, 1], F32, name="eps")
        nc.vector.memset(eps_t[:], EPS)
        dummy = persist.tile([P, 1], F32, name="dummy")

        xch = [persist.tile([P, L], BF16, name=f"x{c}") for c in range(DCH)]
        r_bcast = persist.tile([P, L], F32, name="r_bcast")
        qbar = [persist.tile([P, L], BF16, name=f"qb{i}") for i in range(2)]
        kbar = [persist.tile([P, L], BF16, name=f"kb{i}") for i in range(2)]
        vprime = [persist.tile([P, HPC, HS + 2], BF16, name=f"vp{t}")
                  for t in range(TT)]
        onrm = [persist.tile([P, L], BF16, name=f"on{i}") for i in range(2)]

        # token-major LN stat tiles [128, TT]
        sum_col = persist.tile([P, TT], F32, name="sumc")
        sumsq_col = persist.tile([P, TT], F32, name="sqc")
        mu_col = persist.tile([P, TT], F32, name="muc")
        mu2_col = persist.tile([P, TT], F32, name="mu2c")
        var_col = persist.tile([P, TT], F32, name="varc")
        std_col = persist.tile([P, TT], F32, name="stdc")
        rscr_col = persist.tile([P, TT], F32, name="rscrc")
        r_cols = persist.tile([P, TT], F32, name="rcol")
        nm_col = persist.tile([P, TT], F32, name="nmc")
        m2_col = persist.tile([P, TT], F32, name="m2c")
        stat2 = persist.tile([P, 2, TT], F32, name="stat2")
        m2row_f = persist.tile([1, L], F32, name="m2row_f")
        m2ones = persist.tile([2, L], BF16, name="m2ones")
        r_row = persist.tile([1, L], F32, name="r_row")
        s_bc = [persist.tile([P, FPC], BF16, name=f"sbc{j}")
                for j in range(2)]
        scrow = [persist.tile([1, FPC], BF16, name=f"scr{j}")
                 for j in range(2)]
        vaug16 = [persist.tile([P, FPC], F32, name=f"va{t}")
                  for t in range(TT)]
        ident_t = persist.tile([P, P], F32, name="ident_t")
        statT = persist.tile([32, P], F32, name="statT")

        # ------------- input DMAs -------------
        # sync queue: wq/wk first (needed with the first x^T chunks), then
        # x^T; scalar queue: token-major x first (stats), then wv/wo.
        wp = ctx.enter_context(tc.tile_pool(name="wp", bufs=1))
        wq_t = [wp.tile([P, FPC], BF16, name=f"wq{c}") for c in range(KCH)]
        wk_t = [wp.tile([P, FPC], BF16, name=f"wk{c}") for c in range(KCH)]
        wv_t = [wp.tile([P, FPC], BF16, name=f"wv{c}") for c in range(KCH)]
        wo_t = [wp.tile([P, D], BF16, name=f"wo{ch}") for ch in range(2)]
        for c in range(KCH):
            nc.sync.dma_start(out=wq_t[c][:], in_=wq[P * c:P * (c + 1), :])
            nc.sync.dma_start(out=wk_t[c][:], in_=wk[P * c:P * (c + 1), :])
            if c < DCH:
                nc.sync.dma_start(
                    out=xch[c][:], in_=xT[P * c:P * (c + 1), :])
        for c in range(KCH):
            nc.sync.dma_start(out=wv_t[c][:], in_=wv[P * c:P * (c + 1), :])
        for ch in range(2):
            nc.sync.dma_start(out=wo_t[ch][:], in_=wo[P * ch:P * (ch + 1), :])

        nc.scalar.dma_start(out=ident_t[:], in_=ident[:, :])
        nc.vector.memset(m2ones[:], 1.0)   # row 1 stays ones; row 0 gets m2

        # ------------- phase A: LN stats off the PE -------------
        with ExitStack() as astk:
            xtp = astk.enter_context(tc.tile_pool(name="xtp", bufs=6))
            sqp = astk.enter_context(tc.tile_pool(name="sqp", bufs=3))  # rotates cpy+sq

            def emit_stats(i):
                xtile = xtp.tile([P, D], BF16, name="xtile")
                nc.scalar.dma_start(
                    out=xtile[:], in_=xtok[P * i:P * (i + 1), :])
                cpy = sqp.tile([P, D], BF16, name="cpy")
                nc.scalar.activation(
                    out=cpy[:], in_=xtile[:],
                    func=mybir.ActivationFunctionType.Copy,
                    accum_out=sum_col[:, i:i + 1])
                sq = sqp.tile([P, D], BF16, name="sq")
                nc.scalar.activation(
                    out=sq[:], in_=xtile[:],
                    func=mybir.ActivationFunctionType.Square,
                    accum_out=sumsq_col[:, i:i + 1])

            for i in range(10):
                emit_stats(i)

            # ------------- phase B: projections -------------
            # Q/K chunks 0..7 run as x^T arrives; the c0..7 partials are
            # staged to SBUF so the PSUM banks free up.  The augmented
            # correction attaches later as a tiny K=2 matmul on [m2; ones]
            # plus a DVE scale/add.
            qst = astk.enter_context(tc.tile_pool(name="qst", bufs=1))
            stg = [[[qst.tile([P, 512], F32, name=f"st{m}{d}{s}")
                     for s in range(4)] for d in range(2)] for m in range(2)]
            tpf = astk.enter_context(tc.tile_pool(name="tpf", bufs=3))

            # m0: full 8-bank chunk-outer pass; the stage copies are emitted
            # before the last stats reduces so the DVE frees m0's banks as
            # soon as the matmuls finish
            with ExitStack() as bstk:
                qkp = bstk.enter_context(
                    tc.tile_pool(name="qkp0", bufs=1, space="PSUM"))
                pq = [[qkp.tile([P, 512], F32, name=f"pq{d}{s}")
                       for s in range(4)] for d in range(2)]
                for c in range(DCH):
                    for d, wt in ((0, wq_t), (1, wk_t)):
                        for s in range(4):
                            sl = slice(512 * s, 512 * (s + 1))
                            nc.tensor.matmul(
                                pq[d][s][:], wt[c][:, 0:P], xch[c][:, sl],
                                start=(c == 0), stop=(c == DCH - 1))
                for d in range(2):
                    for s in range(4):
                        nc.vector.tensor_copy(stg[0][d][s][:], pq[d][s][:])
                for i in range(10, TT):
                    emit_stats(i)

            # LN chain, token-major [128, 16].  With r = 1/std the augmented
            # correction reduces to  s_f*m2_t + c_f  where m2 = -mu/std.
            nc.vector.tensor_scalar_mul(mu_col[:], sum_col[:], 1.0 / D)
            nc.vector.tensor_scalar_mul(nm_col[:], sum_col[:], -1.0 / D)
            nc.vector.tensor_mul(mu2_col[:], mu_col[:], mu_col[:])
            nc.vector.scalar_tensor_tensor(
                out=var_col[:], in0=sumsq_col[:], scalar=1.0 / D,
                in1=mu2_col[:], op0=mybir.AluOpType.mult,
                op1=mybir.AluOpType.subtract)
            nc.scalar.activation(
                out=std_col[:], in_=var_col[:],
                func=mybir.ActivationFunctionType.Sqrt,
                bias=eps_t[:], scale=1.0)
            # preload the exp table set right after the last sqrt use
            nc.scalar.activation(
                out=dummy[:], in_=eps_t[:],
                func=mybir.ActivationFunctionType.Exp, scale=1.0)
            nc.vector.reciprocal_approx_accurate(
                out=r_cols[:], in_=std_col[:], scratch=rscr_col[:])
            nc.vector.tensor_mul(m2_col[:], nm_col[:], r_cols[:])
            nc.vector.tensor_copy(stat2[:, 0, :], m2_col[:])
            nc.vector.tensor_copy(stat2[:, 1, :], r_cols[:])

            # broadcast the V' aug weight rows (s, c) across partitions
            nc.vector.tensor_copy(scrow[0][:], wv_t[DCH][0:1, :])
            nc.scalar.dma_start(out=scrow[1][:], in_=wv_t[DCH][1:2, :])
            nc.gpsimd.partition_broadcast(s_bc[0][:], scrow[0][:])
            nc.gpsimd.partition_broadcast(s_bc[1][:], scrow[1][:])


            # m1: two 4-bank waves (x fully resident by now)
            for d, wt in ((0, wq_t), (1, wk_t)):
                with ExitStack() as bstk:
                    qkp = bstk.enter_context(
                        tc.tile_pool(name=f"qkp1{d}", bufs=1, space="PSUM"))
                    pq1 = [qkp.tile([P, 512], F32, name=f"pq{s}")
                           for s in range(4)]
                    for c in range(DCH):
                        for s in range(4):
                            sl = slice(512 * s, 512 * (s + 1))
                            nc.tensor.matmul(
                                pq1[s][:], wt[c][:, P:2 * P], xch[c][:, sl],
                                start=(c == 0), stop=(c == DCH - 1))
                    for s in range(4):
                        nc.vector.tensor_copy(stg[1][d][s][:], pq1[s][:])

            for t in range(TT):
                nc.vector.scalar_tensor_tensor(
                    out=vaug16[t][:], in0=s_bc[0][:],
                    scalar=m2_col[:, t:t + 1], in1=s_bc[1][:],
                    op0=mybir.AluOpType.mult, op1=mybir.AluOpType.add)

            # V' as a continuous 6-deep PSUM rotation (chunk-inner per
            # token tile, no wave barriers); the Q/K finish interleaves into
            # the stream so bar[] is ready well before attention
            with ExitStack() as vstk:
                vps = vstk.enter_context(
                    tc.tile_pool(name="vps", bufs=6, space="PSUM"))
                qkf = vstk.enter_context(
                    tc.tile_pool(name="qkf", bufs=2, space="PSUM"))

                def emit_vtile(t):
                    pv = vps.tile([P, FPC], F32, name="pv")
                    for c in range(DCH):
                        nc.tensor.matmul(
                            pv[:], xch[c][:, P * t:P * (t + 1)], wv_t[c][:],
                            start=(c == 0), stop=(c == DCH - 1))
                    nc.vector.memset(vprime[t][:, :, HS:HS + 1], 1.0)
                    nc.vector.memset(vprime[t][:, :, HS + 1:HS + 2], 0.0)
                    nc.vector.scalar_tensor_tensor(
                        out=vprime[t][:, :, 0:HS],
                        in0=pv[:].rearrange("p (h f) -> p h f", h=HPC),
                        scalar=r_cols[:, t:t + 1],
                        in1=vaug16[t][:].rearrange("p (h f) -> p h f", h=HPC),
                        op0=mybir.AluOpType.mult,
                        op1=mybir.AluOpType.add)

                def emit_qkf(m):
                    for d, wt, bar in ((0, wq_t, qbar), (1, wk_t, kbar)):
                        for s in range(4):
                            sl = slice(512 * s, 512 * (s + 1))
                            pf = qkf.tile([P, 512], F32, name="pf")
                            nc.tensor.matmul(
                                pf[:], wt[DCH][0:2, P * m:P * (m + 1)],
                                m2ones[:, sl], start=True, stop=True)
                            tf = tpf.tile([P, 512], F32, name="tf")
                            nc.vector.tensor_mul(
                                tf[:], stg[m][d][s][:], r_bcast[:, sl])
                            nc.vector.tensor_add(
                                bar[m][:, sl], tf[:], pf[:])

                # cols -> rows via PE transpose (placed here so the
                # tensor stream never stalls on the stats): stat2 [128, 32]
                # -> [32, 128] in a qkf-pool bank, then two 16-descriptor
                # SBUF row gathers
                ptx = qkf.tile([P, 512], F32, name="pf")
                nc.tensor.transpose(
                    ptx[0:32, 0:P], stat2[:].rearrange("p j i -> p (j i)"),
                    ident_t[:])
                nc.vector.tensor_copy(statT[:], ptx[0:32, 0:P])
                nc.scalar.dma_start(
                    out=m2row_f[0:1, :].rearrange("o (i p) -> o i p", i=TT),
                    in_=statT[0:TT, :])
                nc.sync.dma_start(
                    out=r_row[0:1, :].rearrange("o (i p) -> o i p", i=TT),
                    in_=statT[TT:2 * TT, :])
                nc.vector.tensor_copy(m2ones[0:1, :], m2row_f[:])
                nc.gpsimd.partition_broadcast(r_bcast[:], r_row[:])
                for t in range(8):
                    emit_vtile(t)
                emit_qkf(0)
                for t in range(8, TT):
                    emit_vtile(t)
                emit_qkf(1)

        # ------------- phase C: attention + delayed out-proj -------------
        with ExitStack() as cstk:
            epool = cstk.enter_context(tc.tile_pool(name="epool", bufs=6))
            dpool = cstk.enter_context(tc.tile_pool(name="dpool", bufs=6))
            dbcp = cstk.enter_context(tc.tile_pool(name="dbcp", bufs=2))
            ostg = cstk.enter_context(tc.tile_pool(name="ostg", bufs=3))
            spool = cstk.enter_context(
                tc.tile_pool(name="spool", bufs=2, space="PSUM"))
            opool = cstk.enter_context(
                tc.tile_pool(name="opool", bufs=2, space="PSUM"))
            opjp = cstk.enter_context(
                tc.tile_pool(name="opjp", bufs=2, space="PSUM"))

            def emit_outproj(q4):
                for tt in range(QS // P):
                    t = (QS // P) * q4 + tt
                    for s2 in range(2):
                        po = opjp.tile([P, 512], F32, name="po")
                        for ch in range(2):
                            nc.tensor.matmul(
                                po[:], onrm[ch][:, P * t:P * (t + 1)],
                                wo_t[ch][:, 512 * s2:512 * (s2 + 1)],
                                start=(ch == 0), stop=(ch == 1))
                        ot = ostg.tile([P, 512], BF16, name="ot")
                        nc.vector.tensor_copy(ot[:], po[:])
                        oeng = nc.sync if (tt + s2) % 2 == 0 else nc.scalar
                        oeng.dma_start(
                            out=out[P * t:P * (t + 1),
                                    512 * s2:512 * (s2 + 1)],
                            in_=ot[:])

            for q4 in range(NQS):
                qsl = slice(QS * q4, QS * (q4 + 1))
                for pair in range(2):
                    qb, kb = qbar[pair], kbar[pair]
                    # both heads' O^T accumulators; scores for the pair share
                    # one [128, 1024] tile: ho0 -> cols 0:512, ho1 -> 512:1024
                    ops = [opool.tile([HS + 2, QS], F32, name="op")
                           for _ in range(2)]
                    eprev = None
                    for kt in range(KT):
                        ksl = slice(P * kt, P * (kt + 1))
                        sp = spool.tile([P, 2 * QS], F32, name="sp")
                        for ho in range(2):
                            hb = HS * ho
                            nc.tensor.matmul(
                                sp[:, QS * ho:QS * (ho + 1)],
                                kb[hb:hb + HS, ksl], qb[hb:hb + HS, qsl],
                                start=True, stop=True)
                        e = epool.tile([P, 2 * QS], BF16, name="e")
                        nc.scalar.activation(
                            out=e[:], in_=sp[:],
                            func=mybir.ActivationFunctionType.Exp,
                            scale=SCALE)
                        if kt > 0:
                            for ho in range(2):
                                nc.tensor.matmul(
                                    ops[ho][:],
                                    vprime[kt - 1][:, 2 * pair + ho, :],
                                    eprev[:, QS * ho:QS * (ho + 1)],
                                    start=(kt == 1), stop=False)
                        eprev = e
                    for ho in range(2):
                        nc.tensor.matmul(
                            ops[ho][:], vprime[KT - 1][:, 2 * pair + ho, :],
                            eprev[:, QS * ho:QS * (ho + 1)],
                            start=False, stop=True)
                    # normalize (denominator on PSUM partition 64)
                    for ho in range(2):
                        den = dpool.tile([1, QS], F32, name="den")
                        nc.vector.tensor_copy(den[:], ops[ho][HS:HS + 1, :])
                        dinv = dpool.tile([1, QS], F32, name="dinv")
                        dscr = dpool.tile([1, QS], F32, name="dscr")
                        nc.vector.reciprocal_approx_accurate(
                            out=dinv[:], in_=den[:], scratch=dscr[:])
                        dbc = dbcp.tile([HS, QS], F32, name="dbc")
                        nc.gpsimd.partition_broadcast(dbc[:], dinv[:])
                        nc.vector.tensor_mul(
                            onrm[pair][HS * ho:HS * ho + HS, qsl],
                            ops[ho][0:HS, :], dbc[:])
                # previous slab's out-proj: its normalize finished a whole
                # pair ago, so the PE stream doesn't stall on it
                if q4 > 0:
                    emit_outproj(q4 - 1)
            emit_outproj(NQS - 1)

    nc.compile()
    return nc


_NC = None


def _host_weights(W, bias, ln_g, ln_b, rows):
    Wt = W * ln_g[None, :]
    c = W @ ln_b + bias
    s = Wt.sum(axis=1)
    What = np.zeros((KCH * P, FPC), np.float32)
    What[0:D, :] = Wt[rows].T
    What[D, :] = s[rows]
    What[D + 1, :] = c[rows]
    return What


def kernel(x, ln_g, ln_b, Wq, bq, Wk, bk, Wv, bv, Wo, bo):
    global _NC, LAST_RESULTS
    x = np.ascontiguousarray(np.asarray(x, np.float32))
    ln_g = np.asarray(ln_g, np.float32)
    ln_b = np.asarray(ln_b, np.float32)
    Wq, bq = np.asarray(Wq, np.float32), np.asarray(bq, np.float32)
    Wk, bk = np.asarray(Wk, np.float32), np.asarray(bk, np.float32)
    Wv, bv = np.asarray(Wv, np.float32), np.asarray(bv, np.float32)
    Wo, bo = np.asarray(Wo, np.float32), np.asarray(bo, np.float32)

    if _NC is None:
        _NC = _build_nc()

    import ml_dtypes
    bf = ml_dtypes.bfloat16
    in_maps = []
    xt_b = [np.ascontiguousarray(x[b]).astype(bf) for b in range(B)]
    ident_np = np.eye(P, dtype=np.float32)
    xT_b = [np.ascontiguousarray(x[b].T).astype(bf) for b in range(B)]
    for core in range(NCORES):
        b, g = core // HPC, core % HPC
        rows = slice(FPC * g, FPC * (g + 1))
        in_maps.append({
            "xT": xT_b[b],
            "xtok": xt_b[b],
            "ident": ident_np,
            "wq": _host_weights(Wq, bq, ln_g, ln_b, rows).astype(bf),
            "wk": _host_weights(Wk, bk, ln_g, ln_b, rows).astype(bf),
            "wv": _host_weights(Wv, bv, ln_g, ln_b, rows).astype(bf),
            "wo": np.ascontiguousarray(Wo[:, rows].T).astype(bf),
        })

    res = run_bass_kernel_spmd(
        _NC, in_maps, core_ids=list(range(NCORES)),
        trace=bool(int(os.environ.get("KERNEL_TRACE", "0"))),
    )
    LAST_RESULTS = res

    out = np.zeros((B, L, D), np.float32)
    for b in range(B):
        acc = res.results[HPC * b]["out"].astype(np.float32).copy()
        for g in range(1, HPC):
            acc += res.results[HPC * b + g]["out"]
        out[b] = acc + bo[None, :]
    return out


# revision 42
# speedup vs baseline: 1.0235x; 1.0235x over previous
"""Trainium2 Bass kernel for nn_AttnBlock: LayerNorm -> 16-head attention -> out-proj.

Full inputs in, full output out. Sharding: 8 cores = 2 batches x 4 head-groups
(4 heads per core). Each core computes LN + QKV (its 256 feature slice) +
attention for its 4 heads + a partial output projection; the host sums the 4
partials per batch and adds the output bias.

v3 changes vs baseline (335 us):
  - LN stats off the tensor engine: x also arrives token-major (xtok) and the
    stats run on the vector engine while the PE runs Q/K chunks 0..7 (which
    don't depend on the stats). LN chain in token-major [128, 16] form
    (short ops) instead of [1, 2048] rows.
  - Q/K head-pair 0 accumulates chunk-outer across 8 PSUM banks so the PE
    starts as soon as the first x^T chunk lands; the augmented chunk 8
    attaches once the stats bounce completes.
  - attention in 512-token q slabs with BOTH heads of a pair packed into one
    [128, 1024] score tile (ho0 -> cols 0:512, ho1 -> 512:1024; the two K=64
    score matmuls still run concurrently in disjoint PE row groups), so one
    1024-wide exp covers the pair -> half the ScalarE instruction overhead.
  - AV software-pipelined one k-tile behind the scores so the PE never waits
    on the exp.
  - out-proj of slab q runs one slab later (inside slab q+1's PE stream),
    hiding the normalize latency and shrinking the end tail.
  - weights go first on the sync queue, xtok first on the scalar queue, so
    Q/K weights precede x^T and the stats inputs are not starved.
"""

import os
from contextlib import ExitStack

import numpy as np

import concourse.bass as bass
import concourse.tile as tile
from concourse import bacc, mybir
from concourse.bass_utils import run_bass_kernel_spmd

F32 = mybir.dt.float32
BF16 = mybir.dt.bfloat16

B, L, D = 2, 2048, 1024
NH_TOT, HS = 16, 64
NCORES = 8
HPC = 4                  # heads per core
FPC = HPC * HS           # 256 features per core
P = 128
DCH = D // P             # 8 x^T chunks
KCH = DCH + 1            # +1 augmented chunk
QS = 512                 # q slab
NQS = L // QS            # 4
KT = L // P              # 16 k tiles
TT = L // P              # 16 token tiles
EPS = 1e-5
SCALE = float(HS) ** -0.5

LAST_RESULTS = None


def _build_nc():
    nc = bacc.Bacc("TRN2", target_bir_lowering=False, debug=False)

    xT = nc.dram_tensor("xT", [D, L], BF16, kind="ExternalInput").ap()
    xtok = nc.dram_tensor("xtok", [L, D], BF16, kind="ExternalInput").ap()
    wq = nc.dram_tensor("wq", [KCH * P, FPC], BF16, kind="ExternalInput").ap()
    wk = nc.dram_tensor("wk", [KCH * P, FPC], BF16, kind="ExternalInput").ap()
    wv = nc.dram_tensor("wv", [KCH * P, FPC], BF16, kind="ExternalInput").ap()
    wo = nc.dram_tensor("wo", [FPC, D], BF16, kind="ExternalInput").ap()
    ident = nc.dram_tensor("ident", [P, P], F32, kind="ExternalInput").ap()
    out = nc.dram_tensor("out", [L, D], BF16, kind="ExternalOutput").ap()

    with tile.TileContext(nc) as tc, ExitStack() as ctx:
        persist = ctx.enter_context(tc.tile_pool(name="persist", bufs=1))

        # ---------------- persistent tiles ----------------
        eps_t = persist.tile([P, 1], F32, name="eps")
        nc.vector.memset(eps_t[:], EPS)
        dummy = persist.tile([P, 1], F32, name="dummy")

        xch = [persist.tile([P, L], BF16, name=f"x{c}") for c in range(DCH)]
        r_bcast = persist.tile([P, L], F32, name="r_bcast")
        qbar = [persist.tile([P, L], BF16, name=f"qb{i}") for i in range(2)]
        kbar = [persist.tile([P, L], BF16, name=f"kb{i}") for i in range(2)]
        vprime = [persist.tile([P, HPC, HS + 2], BF16, name=f"vp{t}")
                  for t in range(TT)]
        onrm = [persist.tile([P, L], BF16, name=f"on{i}") for i in range(2)]

        # token-major LN stat tiles [128, TT]
        sum_col = persist.tile([P, TT], F32, name="sumc")
        sumsq_col = persist.tile([P, TT], F32, name="sqc")
        mu_col = persist.tile([P, TT], F32, name="muc")
        mu2_col = persist.tile([P, TT], F32, name="mu2c")
        var_col = persist.tile([P, TT], F32, name="varc")
        std_col = persist.tile([P, TT], F32, name="stdc")
        rscr_col = persist.tile([P, TT], F32, name="rscrc")
        r_cols = persist.tile([P, TT], F32, name="rcol")
        nm_col = persist.tile([P, TT], F32, name="nmc")
        m2_col = persist.tile([P, TT], F32, name="m2c")
        stat2 = persist.tile([P, 2, TT], F32, name="stat2")
        m2row_f = persist.tile([1, L], F32, name="m2row_f")
        m2ones = persist.tile([2, L], BF16, name="m2ones")
        r_row = persist.tile([1, L], F32, name="r_row")
        s_bc = [persist.tile([P, FPC], BF16, name=f"sbc{j}")
                for j in range(2)]
        scrow = [persist.tile([1, FPC], BF16, name=f"scr{j}")
                 for j in range(2)]
        vaug16 = [persist.tile([P, FPC], F32, name=f"va{t}")
                  for t in range(TT)]
        ident_t = persist.tile([P, P], F32, name="ident_t")
        statT = persist.tile([32, P], F32, name="statT")

        # ------------- input DMAs -------------
        # sync queue: wq/wk first (needed with the first x^T chunks), then
        # x^T; scalar queue: token-major x first (stats), then wv/wo.
        wp = ctx.enter_context(tc.tile_pool(name="wp", bufs=1))
        wq_t = [wp.tile([P, FPC], BF16, name=f"wq{c}") for c in range(KCH)]
        wk_t = [wp.tile([P, FPC], BF16, name=f"wk{c}") for c in range(KCH)]
        wv_t = [wp.tile([P, FPC], BF16, name=f"wv{c}") for c in range(KCH)]
        wo_t = [wp.tile([P, D], BF16, name=f"wo{ch}") for ch in range(2)]
        for c in range(KCH):
            nc.sync.dma_start(out=wq_t[c][:], in_=wq[P * c:P * (c + 1), :])
            nc.sync.dma_start(out=wk_t[c][:], in_=wk[P * c:P * (c + 1), :])
            if c < DCH:
                nc.sync.dma_start(
                    out=xch[c][:], in_=xT[P * c:P * (c + 1), :])
        for c in range(KCH):
            nc.sync.dma_start(out=wv_t[c][:], in_=wv[P * c:P * (c + 1), :])
        for ch in range(2):
            nc.sync.dma_start(out=wo_t[ch][:], in_=wo[P * ch:P * (ch + 1), :])

        nc.scalar.dma_start(out=ident_t[:], in_=ident[:, :])
        nc.vector.memset(m2ones[:], 1.0)   # row 1 stays ones; row 0 gets m2

        # ------------- phase A: LN stats off the PE -------------
        with ExitStack() as astk:
            xtp = astk.enter_context(tc.tile_pool(name="xtp", bufs=6))
            sqp = astk.enter_context(tc.tile_pool(name="sqp", bufs=3))  # rotates cpy+sq

            def emit_stats(i):
                xtile = xtp.tile([P, D], BF16, name="xtile")
                nc.scalar.dma_start(
                    out=xtile[:], in_=xtok[P * i:P * (i + 1), :])
                cpy = sqp.tile([P, D], BF16, name="cpy")
                nc.scalar.activation(
                    out=cpy[:], in_=xtile[:],
                    func=mybir.ActivationFunctionType.Copy,
                    accum_out=sum_col[:, i:i + 1])
                sq = sqp.tile([P, D], BF16, name="sq")
                nc.scalar.activation(
                    out=sq[:], in_=xtile[:],
                    func=mybir.ActivationFunctionType.Square,
                    accum_out=sumsq_col[:, i:i + 1])

            for i in range(10):
                emit_stats(i)

            # ------------- phase B: projections -------------
            # Q/K chunks 0..7 run as x^T arrives; the c0..7 partials are
            # staged to SBUF so the PSUM banks free up.  The augmented
            # correction attaches later as a tiny K=2 matmul on [m2; ones]
            # plus a DVE scale/add.
            qst = astk.enter_context(tc.tile_pool(name="qst", bufs=1))
            stg = [[[qst.tile([P, 512], F32, name=f"st{m}{d}{s}")
                     for s in range(4)] for d in range(2)] for m in range(2)]
            tpf = astk.enter_context(tc.tile_pool(name="tpf", bufs=3))

            # m0: full 8-bank chunk-outer pass; the stage copies are emitted
            # before the last stats reduces so the DVE frees m0's banks as
            # soon as the matmuls finish
            with ExitStack() as bstk:
                qkp = bstk.enter_context(
                    tc.tile_pool(name="qkp0", bufs=1, space="PSUM"))
                pq = [[qkp.tile([P, 512], F32, name=f"pq{d}{s}")
                       for s in range(4)] for d in range(2)]
                for c in range(DCH):
                    for d, wt in ((0, wq_t), (1, wk_t)):
                        for s in range(4):
                            sl = slice(512 * s, 512 * (s + 1))
                            nc.tensor.matmul(
                                pq[d][s][:], wt[c][:, 0:P], xch[c][:, sl],
                                start=(c == 0), stop=(c == DCH - 1))
                for d in range(2):
                    for s in range(4):
                        nc.vector.tensor_copy(stg[0][d][s][:], pq[d][s][:])
                for i in range(10, TT):
                    emit_stats(i)

            # LN chain, token-major [128, 16].  With r = 1/std the augmented
            # correction reduces to  s_f*m2_t + c_f  where m2 = -mu/std.
            nc.vector.tensor_scalar_mul(mu_col[:], sum_col[:], 1.0 / D)
            nc.vector.tensor_scalar_mul(nm_col[:], sum_col[:], -1.0 / D)
            nc.vector.tensor_mul(mu2_col[:], mu_col[:], mu_col[:])
            nc.vector.scalar_tensor_tensor(
                out=var_col[:], in0=sumsq_col[:], scalar=1.0 / D,
                in1=mu2_col[:], op0=mybir.AluOpType.mult,
                op1=mybir.AluOpType.subtract)
            nc.scalar.activation(
                out=std_col[:], in_=var_col[:],
                func=mybir.ActivationFunctionType.Sqrt,
                bias=eps_t[:], scale=1.0)
            # preload the exp table set right after the last sqrt use
            nc.scalar.activation(
                out=dummy[:], in_=eps_t[:],
                func=mybir.ActivationFunctionType.Exp, scale=1.0)
            nc.vector.reciprocal_approx_accurate(
                out=r_cols[:], in_=std_col[:], scratch=rscr_col[:])
            nc.vector.tensor_mul(m2_col[:], nm_col[:], r_cols[:])
            nc.vector.tensor_copy(stat2[:, 0, :], m2_col[:])
            nc.vector.tensor_copy(stat2[:, 1, :], r_cols[:])

            # broadcast the V' aug weight rows (s, c) across partitions
            nc.vector.tensor_copy(scrow[0][:], wv_t[DCH][0:1, :])
            nc.scalar.dma_start(out=scrow[1][:], in_=wv_t[DCH][1:2, :])
            nc.gpsimd.partition_broadcast(s_bc[0][:], scrow[0][:])
            nc.gpsimd.partition_broadcast(s_bc[1][:], scrow[1][:])


            # m1: two 4-bank waves (x fully resident by now)
            for d, wt in ((0, wq_t), (1, wk_t)):
                with ExitStack() as bstk:
                    qkp = bstk.enter_context(
                        tc.tile_pool(name=f"qkp1{d}", bufs=1, space="PSUM"))
                    pq1 = [qkp.tile([P, 512], F32, name=f"pq{s}")
                           for s in range(4)]
                    for c in range(DCH):
                        for s in range(4):
                            sl = slice(512 * s, 512 * (s + 1))
                            nc.tensor.matmul(
                                pq1[s][:], wt[c][:, P:2 * P], xch[c][:, sl],
                                start=(c == 0), stop=(c == DCH - 1))
                    for s in range(4):
                        nc.vector.tensor_copy(stg[1][d][s][:], pq1[s][:])

            for t in range(TT):
                nc.vector.scalar_tensor_tensor(
                    out=vaug16[t][:], in0=s_bc[0][:],
                    scalar=m2_col[:, t:t + 1], in1=s_bc[1][:],
                    op0=mybir.AluOpType.mult, op1=mybir.AluOpType.add)

            # V' as a continuous 6-deep PSUM rotation (chunk-inner per
            # token tile, no wave barriers); the Q/K finish interleaves into
            # the stream so bar[] is ready well before attention
            with ExitStack() as vstk:
                vps = vstk.enter_context(
                    tc.tile_pool(name="vps", bufs=6, space="PSUM"))
                qkf = vstk.enter_context(
                    tc.tile_pool(name="qkf", bufs=2, space="PSUM"))

                def emit_vtile(t):
                    pv = vps.tile([P, FPC], F32, name="pv")
                    for c in range(DCH):
                        nc.tensor.matmul(
                            pv[:], xch[c][:, P * t:P * (t + 1)], wv_t[c][:],
                            start=(c == 0), stop=(c == DCH - 1))
                    nc.vector.memset(vprime[t][:, :, HS:HS + 1], 1.0)
                    nc.vector.memset(vprime[t][:, :, HS + 1:HS + 2], 0.0)
                    nc.vector.scalar_tensor_tensor(
                        out=vprime[t][:, :, 0:HS],
                        in0=pv[:].rearrange("p (h f) -> p h f", h=HPC),
                        scalar=r_cols[:, t:t + 1],
                        in1=vaug16[t][:].rearrange("p (h f) -> p h f", h=HPC),
                        op0=mybir.AluOpType.mult,
                        op1=mybir.AluOpType.add)

                def emit_qkf(m):
                    for d, wt, bar in ((0, wq_t, qbar), (1, wk_t, kbar)):
                        for s in range(4):
                            sl = slice(512 * s, 512 * (s + 1))
                            pf = qkf.tile([P, 512], F32, name="pf")
                            nc.tensor.matmul(
                                pf[:], wt[DCH][0:2, P * m:P * (m + 1)],
                                m2ones[:, sl], start=True, stop=True)
                            tf = tpf.tile([P, 512], F32, name="tf")
                            nc.vector.tensor_mul(
                                tf[:], stg[m][d][s][:], r_bcast[:, sl])
                            nc.vector.tensor_add(
                                bar[m][:, sl], tf[:], pf[:])

                # cols -> rows via PE transpose (placed here so the
                # tensor stream never stalls on the stats): stat2 [128, 32]
                # -> [32, 128] in a qkf-pool bank, then two 16-descriptor
                # SBUF row gathers
                ptx = qkf.tile([P, 512], F32, name="pf")
                nc.tensor.transpose(
                    ptx[0:32, 0:P], stat2[:].rearrange("p j i -> p (j i)"),
                    ident_t[:])
                nc.vector.tensor_copy(statT[:], ptx[0:32, 0:P])
                nc.scalar.dma_start(
                    out=m2row_f[0:1, :].rearrange("o (i p) -> o i p", i=TT),
                    in_=statT[0:TT, :])
                nc.sync.dma_start(
                    out=r_row[0:1, :].rearrange("o (i p) -> o i p", i=TT),
                    in_=statT[TT:2 * TT, :])
                nc.vector.tensor_copy(m2ones[0:1, :], m2row_f[:])
                nc.gpsimd.partition_broadcast(r_bcast[:], r_row[:])
                for t in range(8):
                    emit_vtile(t)
                emit_qkf(0)
                for t in range(8, TT):
                    emit_vtile(t)
                emit_qkf(1)

        # ------------- phase C: attention + delayed out-proj -------------
        with ExitStack() as cstk:
            epool = cstk.enter_context(tc.tile_pool(name="epool", bufs=6))
            dpool = cstk.enter_context(tc.tile_pool(name="dpool", bufs=6))
            dbcp = cstk.enter_context(tc.tile_pool(name="dbcp", bufs=2))
            ostg = cstk.enter_context(tc.tile_pool(name="ostg", bufs=3))
            spool = cstk.enter_context(
                tc.tile_pool(name="spool", bufs=2, space="PSUM"))
            opool = cstk.enter_context(
                tc.tile_pool(name="opool", bufs=2, space="PSUM"))
            opjp = cstk.enter_context(
                tc.tile_pool(name="opjp", bufs=2, space="PSUM"))

            def emit_outproj(q4):
                for tt in range(QS // P):
                    t = (QS // P) * q4 + tt
                    for s2 in range(2):
                        po = opjp.tile([P, 512], F32, name="po")
                        for ch in range(2):
                            nc.tensor.matmul(
                                po[:], onrm[ch][:, P * t:P * (t + 1)],
                                wo_t[ch][:, 512 * s2:512 * (s2 + 1)],
                                start=(ch == 0), stop=(ch == 1))
                        ot = ostg.tile([P, 512], BF16, name="ot")
                        nc.vector.tensor_copy(ot[:], po[:])
                        nc.sync.dma_start(
                            out=out[P * t:P * (t + 1),
                                    512 * s2:512 * (s2 + 1)],
                            in_=ot[:])

            for q4 in range(NQS):
                qsl = slice(QS * q4, QS * (q4 + 1))
                for pair in range(2):
                    qb, kb = qbar[pair], kbar[pair]
                    # both heads' O^T accumulators; scores for the pair share
                    # one [128, 1024] tile: ho0 -> cols 0:512, ho1 -> 512:1024
                    ops = [opool.tile([HS + 2, QS], F32, name="op")
                           for _ in range(2)]
                    eprev = None
                    for kt in range(KT):
                        ksl = slice(P * kt, P * (kt + 1))
                        sp = spool.tile([P, 2 * QS], F32, name="sp")
                        for ho in range(2):
                            hb = HS * ho
                            nc.tensor.matmul(
                                sp[:, QS * ho:QS * (ho + 1)],
                                kb[hb:hb + HS, ksl], qb[hb:hb + HS, qsl],
                                start=True, stop=True)
                        e = epool.tile([P, 2 * QS], BF16, name="e")
                        nc.scalar.activation(
                            out=e[:], in_=sp[:],
                            func=mybir.ActivationFunctionType.Exp,
                            scale=SCALE)
                        if kt > 0:
                            for ho in range(2):
                                nc.tensor.matmul(
                                    ops[ho][:],
                                    vprime[kt - 1][:, 2 * pair + ho, :],
                                    eprev[:, QS * ho:QS * (ho + 1)],
                                    start=(kt == 1), stop=False)
                        eprev = e
                    for ho in range(2):
                        nc.tensor.matmul(
                            ops[ho][:], vprime[KT - 1][:, 2 * pair + ho, :],
                            eprev[:, QS * ho:QS * (ho + 1)],
                            start=False, stop=True)
                    # normalize (denominator on PSUM partition 64)
                    for ho in range(2):
                        den = dpool.tile([1, QS], F32, name="den")
                        nc.vector.tensor_copy(den[:], ops[ho][HS:HS + 1, :])
                        dinv = dpool.tile([1, QS], F32, name="dinv")
                        dscr = dpool.tile([1, QS], F32, name="dscr")
                        nc.vector.reciprocal_approx_accurate(
                            out=dinv[:], in_=den[:], scratch=dscr[:])
                        dbc = dbcp.tile([HS, QS], F32, name="dbc")
                        nc.gpsimd.partition_broadcast(dbc[:], dinv[:])
                        nc.vector.tensor_mul(
                            onrm[pair][HS * ho:HS * ho + HS, qsl],
                            ops[ho][0:HS, :], dbc[:])
                # previous slab's out-proj: its normalize finished a whole
                # pair ago, so the PE stream doesn't stall on it
                if q4 > 0:
                    emit_outproj(q4 - 1)
            emit_outproj(NQS - 1)

    nc.compile()
    return nc


_NC = None


def _host_weights(W, bias, ln_g, ln_b, rows):
    Wt = W * ln_g[None, :]
    c = W @ ln_b + bias
    s = Wt.sum(axis=1)
    What = np.zeros((KCH * P, FPC), np.float32)
    What[0:D, :] = Wt[rows].T
    What[D, :] = s[rows]
    What[D + 1, :] = c[rows]
    return What


def kernel(x, ln_g, ln_b, Wq, bq, Wk, bk, Wv, bv, Wo, bo):
    global _NC, LAST_RESULTS
    x = np.ascontiguousarray(np.asarray(x, np.float32))
    ln_g = np.asarray(ln_g, np.float32)
    ln_b = np.asarray(ln_b, np.float32)
    Wq, bq = np.asarray(Wq, np.float32), np.asarray(bq, np.float32)
    Wk, bk = np.asarray(Wk, np.float32), np.asarray(bk, np.float32)
    Wv, bv = np.asarray(Wv, np.float32), np.asarray(bv, np.float32)
    Wo, bo = np.asarray(Wo, np.float32), np.asarray(bo, np.float32)

    if _NC is None:
        _NC = _build_nc()

    import ml_dtypes
    bf = ml_dtypes.bfloat16
    in_maps = []
    xt_b = [np.ascontiguousarray(x[b]).astype(bf) for b in range(B)]
    ident_np = np.eye(P, dtype=np.float32)
    xT_b = [np.ascontiguousarray(x[b].T).astype(bf) for b in range(B)]
    for core in range(NCORES):
        b, g = core // HPC, core % HPC
        rows = slice(FPC * g, FPC * (g + 1))
        in_maps.append({
            "xT": xT_b[b],
            "xtok": xt_b[b],
            "ident": ident_np,
            "wq": _host_weights(Wq, bq, ln_g, ln_b, rows).astype(bf),
            "wk": _host_weights(Wk, bk, ln_g, ln_b, rows).astype(bf),
            "wv": _host_weights(Wv, bv, ln_g, ln_b, rows).astype(bf),
            "wo": np.ascontiguousarray(Wo[:, rows].T).astype(bf),
        })

    res = run_bass_kernel_spmd(
        _NC, in_maps, core_ids=list(range(NCORES)),
        trace=bool(int(os.environ.get("KERNEL_TRACE", "0"))),
    )
    LAST_RESULTS = res

    out = np.zeros((B, L, D), np.float32)
    for b in range(B):
        acc = res.results[HPC * b]["out"].astype(np.float32).copy()
        for g in range(1, HPC):
            acc += res.results[HPC * b + g]["out"]
        out[b] = acc + bo[None, :]
    return out


# revision 43
# speedup vs baseline: 1.0378x; 1.0140x over previous
"""Trainium2 Bass kernel for nn_AttnBlock: LayerNorm -> 16-head attention -> out-proj.

Full inputs in, full output out. Sharding: 8 cores = 2 batches x 4 head-groups
(4 heads per core). Each core computes LN + QKV (its 256 feature slice) +
attention for its 4 heads + a partial output projection; the host sums the 4
partials per batch and adds the output bias.

v3 changes vs baseline (335 us):
  - LN stats off the tensor engine: x also arrives token-major (xtok) and the
    stats run on the vector engine while the PE runs Q/K chunks 0..7 (which
    don't depend on the stats). LN chain in token-major [128, 16] form
    (short ops) instead of [1, 2048] rows.
  - Q/K head-pair 0 accumulates chunk-outer across 8 PSUM banks so the PE
    starts as soon as the first x^T chunk lands; the augmented chunk 8
    attaches once the stats bounce completes.
  - attention in 512-token q slabs with BOTH heads of a pair packed into one
    [128, 1024] score tile (ho0 -> cols 0:512, ho1 -> 512:1024; the two K=64
    score matmuls still run concurrently in disjoint PE row groups), so one
    1024-wide exp covers the pair -> half the ScalarE instruction overhead.
  - AV software-pipelined one k-tile behind the scores so the PE never waits
    on the exp.
  - out-proj of slab q runs one slab later (inside slab q+1's PE stream),
    hiding the normalize latency and shrinking the end tail.
  - weights go first on the sync queue, xtok first on the scalar queue, so
    Q/K weights precede x^T and the stats inputs are not starved.
"""

import os
from contextlib import ExitStack

import numpy as np

import concourse.bass as bass
import concourse.tile as tile
from concourse import bacc, mybir
from concourse.bass_utils import run_bass_kernel_spmd

F32 = mybir.dt.float32
BF16 = mybir.dt.bfloat16

B, L, D = 2, 2048, 1024
NH_TOT, HS = 16, 64
NCORES = 8
HPC = 4                  # heads per core
FPC = HPC * HS           # 256 features per core
P = 128
DCH = D // P             # 8 x^T chunks
KCH = DCH + 1            # +1 augmented chunk
QS = 512                 # q slab
NQS = L // QS            # 4
KT = L // P              # 16 k tiles
TT = L // P              # 16 token tiles
EPS = 1e-5
SCALE = float(HS) ** -0.5

LAST_RESULTS = None


def _build_nc():
    nc = bacc.Bacc("TRN2", target_bir_lowering=False, debug=False)

    xT = nc.dram_tensor("xT", [D, L], BF16, kind="ExternalInput").ap()
    xtok = nc.dram_tensor("xtok", [L, D], BF16, kind="ExternalInput").ap()
    wq = nc.dram_tensor("wq", [KCH * P, FPC], BF16, kind="ExternalInput").ap()
    wk = nc.dram_tensor("wk", [KCH * P, FPC], BF16, kind="ExternalInput").ap()
    wv = nc.dram_tensor("wv", [KCH * P, FPC], BF16, kind="ExternalInput").ap()
    wo = nc.dram_tensor("wo", [FPC, D], BF16, kind="ExternalInput").ap()
    ident = nc.dram_tensor("ident", [P, P], F32, kind="ExternalInput").ap()
    out = nc.dram_tensor("out", [L, D], BF16, kind="ExternalOutput").ap()

    with tile.TileContext(nc) as tc, ExitStack() as ctx:
        persist = ctx.enter_context(tc.tile_pool(name="persist", bufs=1))

        # ---------------- persistent tiles ----------------
        eps_t = persist.tile([P, 1], F32, name="eps")
        nc.vector.memset(eps_t[:], EPS)
        dummy = persist.tile([P, 1], F32, name="dummy")

        xch = [persist.tile([P, L], BF16, name=f"x{c}") for c in range(DCH)]
        r_bcast = persist.tile([P, L], F32, name="r_bcast")
        qbar = [persist.tile([P, L], BF16, name=f"qb{i}") for i in range(2)]
        kbar = [persist.tile([P, L], BF16, name=f"kb{i}") for i in range(2)]
        vprime = [persist.tile([P, HPC, HS + 2], BF16, name=f"vp{t}")
                  for t in range(TT)]
        onrm = [persist.tile([P, L], BF16, name=f"on{i}") for i in range(2)]

        # token-major LN stat tiles [128, TT]
        sum_col = persist.tile([P, TT], F32, name="sumc")
        sumsq_col = persist.tile([P, TT], F32, name="sqc")
        mu_col = persist.tile([P, TT], F32, name="muc")
        mu2_col = persist.tile([P, TT], F32, name="mu2c")
        var_col = persist.tile([P, TT], F32, name="varc")
        std_col = persist.tile([P, TT], F32, name="stdc")
        rscr_col = persist.tile([P, TT], F32, name="rscrc")
        r_cols = persist.tile([P, TT], F32, name="rcol")
        nm_col = persist.tile([P, TT], F32, name="nmc")
        m2_col = persist.tile([P, TT], F32, name="m2c")
        stat2 = persist.tile([P, 2, TT], F32, name="stat2")
        m2row_f = persist.tile([1, L], F32, name="m2row_f")
        m2ones = persist.tile([2, L], BF16, name="m2ones")
        r_row = persist.tile([1, L], F32, name="r_row")
        s_bc = [persist.tile([P, FPC], BF16, name=f"sbc{j}")
                for j in range(2)]
        scrow = [persist.tile([1, FPC], BF16, name=f"scr{j}")
                 for j in range(2)]
        vaug16 = [persist.tile([P, FPC], F32, name=f"va{t}")
                  for t in range(TT)]
        ident_t = persist.tile([P, P], F32, name="ident_t")
        statT = persist.tile([32, P], F32, name="statT")

        # ------------- input DMAs -------------
        # sync queue: wq/wk first (needed with the first x^T chunks), then
        # x^T; scalar queue: token-major x first (stats), then wv/wo.
        wp = ctx.enter_context(tc.tile_pool(name="wp", bufs=1))
        wq_t = [wp.tile([P, FPC], BF16, name=f"wq{c}") for c in range(KCH)]
        wk_t = [wp.tile([P, FPC], BF16, name=f"wk{c}") for c in range(KCH)]
        wv_t = [wp.tile([P, FPC], BF16, name=f"wv{c}") for c in range(KCH)]
        wo_t = [wp.tile([P, D], BF16, name=f"wo{ch}") for ch in range(2)]
        for c in range(KCH):
            nc.sync.dma_start(out=wq_t[c][:], in_=wq[P * c:P * (c + 1), :])
            nc.sync.dma_start(out=wk_t[c][:], in_=wk[P * c:P * (c + 1), :])
            if c < DCH:
                nc.sync.dma_start(
                    out=xch[c][:], in_=xT[P * c:P * (c + 1), :])
        for c in range(KCH):
            nc.sync.dma_start(out=wv_t[c][:], in_=wv[P * c:P * (c + 1), :])
        for ch in range(2):
            nc.sync.dma_start(out=wo_t[ch][:], in_=wo[P * ch:P * (ch + 1), :])

        nc.scalar.dma_start(out=ident_t[:], in_=ident[:, :])
        nc.vector.memset(m2ones[:], 1.0)   # row 1 stays ones; row 0 gets m2

        # ------------- phase A: LN stats off the PE -------------
        with ExitStack() as astk:
            xtp = astk.enter_context(tc.tile_pool(name="xtp", bufs=6))
            sqp = astk.enter_context(tc.tile_pool(name="sqp", bufs=3))  # rotates cpy+sq

            def emit_stats(i):
                xtile = xtp.tile([P, D], BF16, name="xtile")
                nc.scalar.dma_start(
                    out=xtile[:], in_=xtok[P * i:P * (i + 1), :])
                cpy = sqp.tile([P, D], BF16, name="cpy")
                nc.scalar.activation(
                    out=cpy[:], in_=xtile[:],
                    func=mybir.ActivationFunctionType.Copy,
                    accum_out=sum_col[:, i:i + 1])
                sq = sqp.tile([P, D], BF16, name="sq")
                nc.scalar.activation(
                    out=sq[:], in_=xtile[:],
                    func=mybir.ActivationFunctionType.Square,
                    accum_out=sumsq_col[:, i:i + 1])

            for i in range(10):
                emit_stats(i)

            # ------------- phase B: projections -------------
            # Q/K chunks 0..7 run as x^T arrives; the c0..7 partials are
            # staged to SBUF so the PSUM banks free up.  The augmented
            # correction attaches later as a tiny K=2 matmul on [m2; ones]
            # plus a DVE scale/add.
            qst = astk.enter_context(tc.tile_pool(name="qst", bufs=1))
            stg = [[[qst.tile([P, 512], F32, name=f"st{m}{d}{s}")
                     for s in range(4)] for d in range(2)] for m in range(2)]
            tpf = astk.enter_context(tc.tile_pool(name="tpf", bufs=3))

            # m0: full 8-bank chunk-outer pass; the stage copies are emitted
            # before the last stats reduces so the DVE frees m0's banks as
            # soon as the matmuls finish
            with ExitStack() as bstk:
                qkp = bstk.enter_context(
                    tc.tile_pool(name="qkp0", bufs=1, space="PSUM"))
                pq = [[qkp.tile([P, 512], F32, name=f"pq{d}{s}")
                       for s in range(4)] for d in range(2)]
                for c in range(DCH):
                    for d, wt in ((0, wq_t), (1, wk_t)):
                        for s in range(4):
                            sl = slice(512 * s, 512 * (s + 1))
                            nc.tensor.matmul(
                                pq[d][s][:], wt[c][:, 0:P], xch[c][:, sl],
                                start=(c == 0), stop=(c == DCH - 1))
                for d in range(2):
                    for s in range(4):
                        nc.vector.tensor_copy(stg[0][d][s][:], pq[d][s][:])
                for i in range(10, TT):
                    emit_stats(i)

            # LN chain, token-major [128, 16].  With r = 1/std the augmented
            # correction reduces to  s_f*m2_t + c_f  where m2 = -mu/std.
            nc.vector.tensor_scalar_mul(mu_col[:], sum_col[:], 1.0 / D)
            nc.vector.tensor_scalar_mul(nm_col[:], sum_col[:], -1.0 / D)
            nc.vector.tensor_mul(mu2_col[:], mu_col[:], mu_col[:])
            nc.vector.scalar_tensor_tensor(
                out=var_col[:], in0=sumsq_col[:], scalar=1.0 / D,
                in1=mu2_col[:], op0=mybir.AluOpType.mult,
                op1=mybir.AluOpType.subtract)
            nc.scalar.activation(
                out=std_col[:], in_=var_col[:],
                func=mybir.ActivationFunctionType.Sqrt,
                bias=eps_t[:], scale=1.0)
            # preload the exp table set right after the last sqrt use
            nc.scalar.activation(
                out=dummy[:], in_=eps_t[:],
                func=mybir.ActivationFunctionType.Exp, scale=1.0)
            nc.vector.reciprocal_approx_accurate(
                out=r_cols[:], in_=std_col[:], scratch=rscr_col[:])
            nc.vector.tensor_mul(m2_col[:], nm_col[:], r_cols[:])
            nc.vector.tensor_copy(stat2[:, 0, :], m2_col[:])
            nc.vector.tensor_copy(stat2[:, 1, :], r_cols[:])

            # broadcast the V' aug weight rows (s, c) across partitions
            nc.vector.tensor_copy(scrow[0][:], wv_t[DCH][0:1, :])
            nc.scalar.dma_start(out=scrow[1][:], in_=wv_t[DCH][1:2, :])
            nc.gpsimd.partition_broadcast(s_bc[0][:], scrow[0][:])
            nc.gpsimd.partition_broadcast(s_bc[1][:], scrow[1][:])


            # m1: two 4-bank waves (x fully resident by now)
            for d, wt in ((0, wq_t), (1, wk_t)):
                with ExitStack() as bstk:
                    qkp = bstk.enter_context(
                        tc.tile_pool(name=f"qkp1{d}", bufs=1, space="PSUM"))
                    pq1 = [qkp.tile([P, 512], F32, name=f"pq{s}")
                           for s in range(4)]
                    for c in range(DCH):
                        for s in range(4):
                            sl = slice(512 * s, 512 * (s + 1))
                            nc.tensor.matmul(
                                pq1[s][:], wt[c][:, P:2 * P], xch[c][:, sl],
                                start=(c == 0), stop=(c == DCH - 1))
                    for s in range(4):
                        nc.vector.tensor_copy(stg[1][d][s][:], pq1[s][:])

            for t in range(TT):
                nc.vector.scalar_tensor_tensor(
                    out=vaug16[t][:], in0=s_bc[0][:],
                    scalar=m2_col[:, t:t + 1], in1=s_bc[1][:],
                    op0=mybir.AluOpType.mult, op1=mybir.AluOpType.add)

            # V' as a continuous 6-deep PSUM rotation (chunk-inner per
            # token tile, no wave barriers); the Q/K finish interleaves into
            # the stream so bar[] is ready well before attention
            with ExitStack() as vstk:
                vps = vstk.enter_context(
                    tc.tile_pool(name="vps", bufs=6, space="PSUM"))
                qkf = vstk.enter_context(
                    tc.tile_pool(name="qkf", bufs=2, space="PSUM"))

                def emit_vtile(t):
                    pv = vps.tile([P, FPC], F32, name="pv")
                    for c in range(DCH):
                        nc.tensor.matmul(
                            pv[:], xch[c][:, P * t:P * (t + 1)], wv_t[c][:],
                            start=(c == 0), stop=(c == DCH - 1))
                    nc.vector.memset(vprime[t][:, :, HS:HS + 1], 1.0)
                    nc.vector.memset(vprime[t][:, :, HS + 1:HS + 2], 0.0)
                    nc.vector.scalar_tensor_tensor(
                        out=vprime[t][:, :, 0:HS],
                        in0=pv[:].rearrange("p (h f) -> p h f", h=HPC),
                        scalar=r_cols[:, t:t + 1],
                        in1=vaug16[t][:].rearrange("p (h f) -> p h f", h=HPC),
                        op0=mybir.AluOpType.mult,
                        op1=mybir.AluOpType.add)

                def emit_qkf(m):
                    for d, wt, bar in ((0, wq_t, qbar), (1, wk_t, kbar)):
                        for s in range(4):
                            sl = slice(512 * s, 512 * (s + 1))
                            pf = qkf.tile([P, 512], F32, name="pf")
                            nc.tensor.matmul(
                                pf[:], wt[DCH][0:2, P * m:P * (m + 1)],
                                m2ones[:, sl], start=True, stop=True)
                            tf = tpf.tile([P, 512], F32, name="tf")
                            nc.vector.tensor_mul(
                                tf[:], stg[m][d][s][:], r_bcast[:, sl])
                            nc.vector.tensor_add(
                                bar[m][:, sl], tf[:], pf[:])

                # cols -> rows via PE transpose (placed here so the
                # tensor stream never stalls on the stats): stat2 [128, 32]
                # -> [32, 128] in a qkf-pool bank, then two 16-descriptor
                # SBUF row gathers
                ptx = qkf.tile([P, 512], F32, name="pf")
                nc.tensor.transpose(
                    ptx[0:32, 0:P], stat2[:].rearrange("p j i -> p (j i)"),
                    ident_t[:])
                nc.vector.tensor_copy(statT[:], ptx[0:32, 0:P])
                nc.scalar.dma_start(
                    out=m2row_f[0:1, :].rearrange("o (i p) -> o i p", i=TT),
                    in_=statT[0:TT, :])
                nc.sync.dma_start(
                    out=r_row[0:1, :].rearrange("o (i p) -> o i p", i=TT),
                    in_=statT[TT:2 * TT, :])
                nc.vector.tensor_copy(m2ones[0:1, :], m2row_f[:])
                nc.gpsimd.partition_broadcast(r_bcast[:], r_row[:])
                for t in range(8):
                    emit_vtile(t)
                emit_qkf(0)
                for t in range(8, TT):
                    emit_vtile(t)
                emit_qkf(1)

        # ------------- phase C: attention + delayed out-proj -------------
        with ExitStack() as cstk:
            epool = cstk.enter_context(tc.tile_pool(name="epool", bufs=6))
            dpool = cstk.enter_context(tc.tile_pool(name="dpool", bufs=6))
            dbcp = cstk.enter_context(tc.tile_pool(name="dbcp", bufs=2))
            ostg = cstk.enter_context(tc.tile_pool(name="ostg", bufs=3))
            spool = cstk.enter_context(
                tc.tile_pool(name="spool", bufs=2, space="PSUM"))
            opool = cstk.enter_context(
                tc.tile_pool(name="opool", bufs=2, space="PSUM"))
            opjp = cstk.enter_context(
                tc.tile_pool(name="opjp", bufs=2, space="PSUM"))

            def emit_outproj(q4):
                last = q4 == NQS - 1
                for tt in range(QS // P):
                    t = (QS // P) * q4 + tt
                    for s2 in range(2):
                        po = opjp.tile([P, 512], F32, name="po")
                        for ch in range(2):
                            nc.tensor.matmul(
                                po[:], onrm[ch][:, P * t:P * (t + 1)],
                                wo_t[ch][:, 512 * s2:512 * (s2 + 1)],
                                start=(ch == 0), stop=(ch == 1))
                        ot = ostg.tile([P, 512], BF16, name="ot")
                        nc.vector.tensor_copy(ot[:], po[:])
                        oeng = nc.scalar if (last and s2 == 1) else nc.sync
                        oeng.dma_start(
                            out=out[P * t:P * (t + 1),
                                    512 * s2:512 * (s2 + 1)],
                            in_=ot[:])

            for q4 in range(NQS):
                qsl = slice(QS * q4, QS * (q4 + 1))
                for pair in range(2):
                    qb, kb = qbar[pair], kbar[pair]
                    # both heads' O^T accumulators; scores for the pair share
                    # one [128, 1024] tile: ho0 -> cols 0:512, ho1 -> 512:1024
                    ops = [opool.tile([HS + 2, QS], F32, name="op")
                           for _ in range(2)]
                    eprev = None
                    for kt in range(KT):
                        ksl = slice(P * kt, P * (kt + 1))
                        sp = spool.tile([P, 2 * QS], F32, name="sp")
                        for ho in range(2):
                            hb = HS * ho
                            nc.tensor.matmul(
                                sp[:, QS * ho:QS * (ho + 1)],
                                kb[hb:hb + HS, ksl], qb[hb:hb + HS, qsl],
                                start=True, stop=True)
                        e = epool.tile([P, 2 * QS], BF16, name="e")
                        nc.scalar.activation(
                            out=e[:], in_=sp[:],
                            func=mybir.ActivationFunctionType.Exp,
                            scale=SCALE)
                        if kt > 0:
                            for ho in range(2):
                                nc.tensor.matmul(
                                    ops[ho][:],
                                    vprime[kt - 1][:, 2 * pair + ho, :],
                                    eprev[:, QS * ho:QS * (ho + 1)],
                                    start=(kt == 1), stop=False)
                        eprev = e
                    # final AV + normalize interleaved per ho
                    for ho in range(2):
                        nc.tensor.matmul(
                            ops[ho][:], vprime[KT - 1][:, 2 * pair + ho, :],
                            eprev[:, QS * ho:QS * (ho + 1)],
                            start=False, stop=True)
                        den = dpool.tile([1, QS], F32, name="den")
                        nc.vector.tensor_copy(den[:], ops[ho][HS:HS + 1, :])
                        dinv = dpool.tile([1, QS], F32, name="dinv")
                        dscr = dpool.tile([1, QS], F32, name="dscr")
                        nc.vector.reciprocal_approx_accurate(
                            out=dinv[:], in_=den[:], scratch=dscr[:])
                        dbc = dbcp.tile([HS, QS], F32, name="dbc")
                        nc.gpsimd.partition_broadcast(dbc[:], dinv[:])
                        nc.vector.tensor_mul(
                            onrm[pair][HS * ho:HS * ho + HS, qsl],
                            ops[ho][0:HS, :], dbc[:])
                # previous slab's out-proj: its normalize finished a whole
                # pair ago, so the PE stream doesn't stall on it
                if q4 > 0:
                    emit_outproj(q4 - 1)
            emit_outproj(NQS - 1)

    nc.compile()
    return nc


_NC = None


def _host_weights(W, bias, ln_g, ln_b, rows):
    Wt = W * ln_g[None, :]
    c = W @ ln_b + bias
    s = Wt.sum(axis=1)
    What = np.zeros((KCH * P, FPC), np.float32)
    What[0:D, :] = Wt[rows].T
    What[D, :] = s[rows]
    What[D + 1, :] = c[rows]
    return What


def kernel(x, ln_g, ln_b, Wq, bq, Wk, bk, Wv, bv, Wo, bo):
    global _NC, LAST_RESULTS
    x = np.ascontiguousarray(np.asarray(x, np.float32))
    ln_g = np.asarray(ln_g, np.float32)
    ln_b = np.asarray(ln_b, np.float32)
    Wq, bq = np.asarray(Wq, np.float32), np.asarray(bq, np.float32)
    Wk, bk = np.asarray(Wk, np.float32), np.asarray(bk, np.float32)
    Wv, bv = np.asarray(Wv, np.float32), np.asarray(bv, np.float32)
    Wo, bo = np.asarray(Wo, np.float32), np.asarray(bo, np.float32)

    if _NC is None:
        _NC = _build_nc()

    import ml_dtypes
    bf = ml_dtypes.bfloat16
    in_maps = []
    xt_b = [np.ascontiguousarray(x[b]).astype(bf) for b in range(B)]
    ident_np = np.eye(P, dtype=np.float32)
    xT_b = [np.ascontiguousarray(x[b].T).astype(bf) for b in range(B)]
    for core in range(NCORES):
        b, g = core // HPC, core % HPC
        rows = slice(FPC * g, FPC * (g + 1))
        in_maps.append({
            "xT": xT_b[b],
            "xtok": xt_b[b],
            "ident": ident_np,
            "wq": _host_weights(Wq, bq, ln_g, ln_b, rows).astype(bf),
            "wk": _host_weights(Wk, bk, ln_g, ln_b, rows).astype(bf),
            "wv": _host_weights(Wv, bv, ln_g, ln_b, rows).astype(bf),
            "wo": np.ascontiguousarray(Wo[:, rows].T).astype(bf),
        })

    res = run_bass_kernel_spmd(
        _NC, in_maps, core_ids=list(range(NCORES)),
        trace=bool(int(os.environ.get("KERNEL_TRACE", "0"))),
    )
    LAST_RESULTS = res

    out = np.zeros((B, L, D), np.float32)
    for b in range(B):
        acc = res.results[HPC * b]["out"].astype(np.float32).copy()
        for g in range(1, HPC):
            acc += res.results[HPC * b + g]["out"]
        out[b] = acc + bo[None, :]
    return out
